# revision 19
# baseline (speedup 1.0000x reference)
import sys, functools
import numpy as np

sys.path.insert(0, "/opt/trn_rl_repo")

import concourse.bass as bass
import concourse.bacc as bacc
import concourse.mybir as mybir
from concourse import tile
from concourse.vector_clock import ScopedClock


class _TC(tile.TileContext):
    # the local walrus build allows at most ONE sync-wait per instruction;
    # split the kernel-tail drain's waits across single-wait NOPs
    def _drain_and_barrier(self, tick_clock, wait_clock):
        nc = self.nc
        probe = nc.sync.nop()
        wait_clock.add_sem_waits(probe.ins, ScopedClock({None: tick_clock.global_clock}))
        si = probe.ins.sync_info
        ws = list(si.on_wait) if si and si.on_wait else []
        if len(ws) > 1:
            import concourse.mybir as mybir
            probe.ins.sync_info = mybir.SyncInfo(
                on_wait=[ws[0]], on_update=list(si.on_update or []))
            for w in ws[1:]:
                n2 = nc.sync.nop()
                n2.ins.sync_info = mybir.SyncInfo(on_wait=[w], on_update=[])
        nc.sync.drain()
        nc.all_engine_barrier()
        popped = nc._tile_sem_poison_stack.pop()
        assert popped is self._sem_poison
        nc.clear_and_free_semaphores(list(self.sems.allocated().values()))
        nc.all_engine_barrier()
from concourse.bass_utils import run_bass_kernel_spmd

# ---- problem constants (hardcoded per spec) ----
RR, CC, A, G = 256, 256, 9, 64
N = RR * CC * A            # 589824
NCORES = 8
NC = N // NCORES           # 73728 anchors per core
K = NC // 128              # 576 chunk-columns per core
CW = 8                     # chunks packed per wide tile
NIT = K // CW              # 72 iterations
STRIDE = 16
NEG_OV, POS_OV = 0.3, 0.7
RPN_BATCHSIZE, RPN_FG_FRACTION = 256, 0.5
F32 = mybir.dt.float32
ALU = mybir.AluOpType
ACTF = mybir.ActivationFunctionType


def _base_anchors():
    base = np.array([1, 1, 16, 16], dtype=np.float64) - 1
    w = base[2] - base[0] + 1
    h = base[3] - base[1] + 1
    x_ctr = base[0] + 0.5 * (w - 1)
    y_ctr = base[1] + 0.5 * (h - 1)
    size = w * h
    out = []
    for r in (0.5, 1.0, 2.0):
        ws = np.round(np.sqrt(size / r))
        hs = np.round(ws * r)
        for s in (8, 16, 32):
            wss, hss = ws * s, hs * s
            out.append([x_ctr - 0.5 * (wss - 1), y_ctr - 0.5 * (hss - 1),
                        x_ctr + 0.5 * (wss - 1), y_ctr + 0.5 * (hss - 1)])
    return np.array(out, dtype=np.float32)


@functools.lru_cache(maxsize=1)
def _all_anchors():
    base = _base_anchors()
    sx, sy = np.meshgrid(np.arange(CC, dtype=np.float32) * STRIDE,
                         np.arange(RR, dtype=np.float32) * STRIDE)
    shifts = np.stack([sx.ravel(), sy.ravel(), sx.ravel(), sy.ravel()], axis=1).astype(np.float32)
    return (base[None, :, :] + shifts[:, None, :]).reshape(-1, 4)  # (N,4) f32


@functools.lru_cache(maxsize=1)
def _uniforms():
    import jax
    with jax.default_device(jax.devices("cpu")[0]):
        key = jax.random.key(1)
        k_pos, k_neg = jax.random.split(key)
        u_pos = np.asarray(jax.random.uniform(k_pos, (N,)), dtype=np.float32)
        u_neg = np.asarray(jax.random.uniform(k_neg, (N,)), dtype=np.float32)
    return u_pos, u_neg


def _v(ap):  # (128, CW*G) -> (128, CW, G)
    return ap.rearrange("p (c g) -> p c g", c=CW)


@functools.lru_cache(maxsize=1)
def _build_nc():
    nc = bass.Bass()
    COLS = {}
    off = 0
    for nm, w in [("ax1", K), ("ay1", K), ("ax2", K), ("ay2", K), ("areaa", K),
                  ("gx1r", CW * G), ("gy1r", CW * G), ("gx2r", CW * G), ("gy2r", CW * G),
                  ("areagr", CW * G), ("gidxw", CW * G), ("klane", CW * G), ("meta2", 2)]:
        COLS[nm] = (off, w); off += w
    INW = off
    OCOLS = {"lab": (0, K), "tgt": (K, 4 * K), "rmx": (5 * K, G), "rix": (5 * K + G, G)}
    OUTW = 5 * K + 2 * G
    inp = nc.declare_dram_parameter("inp", [128, INW], F32, isOutput=False)
    outp = nc.declare_dram_parameter("out", [128, OUTW], F32, isOutput=True)
    with _TC(nc) as tc:
        with tc.tile_pool(name="main", bufs=2) as pool, \
             tc.tile_pool(name="cons", bufs=1) as cpool, \
             tc.tile_pool(name="ps", bufs=2, space="PSUM") as ppool:
            # load inputs to SBUF (single DMA)
            inpt = cpool.tile([128, INW], F32, tag="inpt")
            nc.sync.dma_start(inpt[:], inp[:])
            nc.vector.tensor_scalar(inpt[:], inpt[:], 0.0, None, ALU.add)
            sb = {nm: inpt[:, o:o + w] for nm, (o, w) in COLS.items()}
            outt = cpool.tile([128, OUTW], F32, tag="outt")
            ax1, ay1, ax2, ay2 = sb["ax1"], sb["ay1"], sb["ax2"], sb["ay2"]
            areaa, inside = sb["areaa"], cpool.tile([128, K], F32, tag="inside")
            # ---- phase A: inside mask, anchor widths ----
            c1 = pool.tile([128, K], F32, tag="pA")
            c2 = pool.tile([128, K], F32, tag="pB")
            nc.vector.tensor_scalar(c1[:], ax1, 0.0, None, ALU.is_ge)
            nc.vector.tensor_scalar(c2[:], ay1, 0.0, None, ALU.is_ge)
            nc.vector.tensor_tensor(inside[:], c1[:], c2[:], ALU.mult)
            nc.vector.tensor_scalar(c1[:], ax2, sb["meta2"][:, 0:1], None, ALU.is_lt)
            nc.vector.tensor_scalar(c2[:], ay2, sb["meta2"][:, 1:2], None, ALU.is_lt)
            nc.vector.tensor_tensor(c1[:], c1[:], c2[:], ALU.mult)
            nc.vector.tensor_tensor(inside[:], inside[:], c1[:], ALU.mult)
            aw = cpool.tile([128, K], F32, tag="aw")
            ah = cpool.tile([128, K], F32, tag="ah")
            raw = cpool.tile([128, K], F32, tag="raw")
            rah = cpool.tile([128, K], F32, tag="rah")
            nc.vector.scalar_tensor_tensor(aw[:], ax2, 1.0, ax1, ALU.add, ALU.subtract)
            nc.vector.scalar_tensor_tensor(ah[:], ay2, 1.0, ay1, ALU.add, ALU.subtract)
            nc.vector.reciprocal(raw[:], aw[:])
            nc.vector.reciprocal(rah[:], ah[:])

            # ---- bulk result tiles ----
            maxsh = cpool.tile([128, K], F32, tag="maxsh")
            g4 = cpool.tile([128, K, 4], F32, tag="g4")
            runmaxw = cpool.tile([128, CW * G], F32, tag="runmaxw")
            runidxw = cpool.tile([128, CW * G], F32, tag="runidxw")
            nc.vector.memset(runmaxw[:], 0.0)
            nc.vector.memset(runidxw[:], 0.0)
            gx1v, gy1v = _v(sb["gx1r"]), _v(sb["gy1r"])
            gx2v, gy2v = _v(sb["gx2r"]), _v(sb["gy2r"])
            areagv, gidxv = _v(sb["areagr"]), _v(sb["gidxw"])

            # ---- phase B: main loop over 72 wide tiles ----
            for it in range(NIT):
                i0 = it * CW
                S = slice(i0, i0 + CW)
                def bc(plane):  # (128,CW) cols -> (128,CW,G) bcast
                    return plane[:, S].rearrange("p (c o) -> p c o", o=1).broadcast_to((128, CW, G))
                t1 = pool.tile([128, CW * G], F32, tag="t1")
                t2 = pool.tile([128, CW * G], F32, tag="t2")
                iwp = pool.tile([128, CW * G], F32, tag="iwp")
                ihp = pool.tile([128, CW * G], F32, tag="ihp")
                nc.vector.tensor_tensor(_v(t1[:]), gx2v, bc(ax2), ALU.min)
                nc.vector.tensor_tensor(_v(t2[:]), gx1v, bc(ax1), ALU.max)
                nc.vector.scalar_tensor_tensor(t1[:], t1[:], 1.0, t2[:], ALU.add, ALU.subtract)
                nc.vector.tensor_scalar(iwp[:], t1[:], 0.0, None, ALU.max)
                t3 = pool.tile([128, CW * G], F32, tag="t3")
                t4 = pool.tile([128, CW * G], F32, tag="t4")
                nc.vector.tensor_tensor(_v(t3[:]), gy2v, bc(ay2), ALU.min)
                nc.vector.tensor_tensor(_v(t4[:]), gy1v, bc(ay1), ALU.max)
                nc.vector.scalar_tensor_tensor(t3[:], t3[:], 1.0, t4[:], ALU.add, ALU.subtract)
                nc.vector.tensor_scalar(ihp[:], t3[:], 0.0, None, ALU.max)
                inter = pool.tile([128, CW * G], F32, tag="inter")
                nc.vector.tensor_tensor(inter[:], iwp[:], ihp[:], ALU.mult)
                un = pool.tile([128, CW * G], F32, tag="un")
                nc.vector.tensor_tensor(_v(un[:]), areagv, bc(areaa), ALU.add)
                nc.vector.tensor_tensor(un[:], un[:], inter[:], ALU.subtract)
                rec = pool.tile([128, CW * G], F32, tag="rec")
                nc.vector.reciprocal(rec[:], un[:])
                iou = pool.tile([128, CW * G], F32, tag="iou")
                nc.vector.tensor_tensor(iou[:], inter[:], rec[:], ALU.mult)
                s2 = pool.tile([128, CW * G], F32, tag="s2")
                nc.vector.tensor_scalar(iou[:], iou[:], 1.0, None, ALU.add)
                nc.vector.tensor_tensor(_v(s2[:]), _v(iou[:]), bc(inside[:]), ALU.mult)
                # per-gt running argmax
                m = pool.tile([128, CW * G], mybir.dt.uint8, tag="m")
                nc.vector.tensor_tensor(m[:], s2[:], runmaxw[:], ALU.is_gt)
                nc.vector.select(runmaxw[:], m[:], s2[:], runmaxw[:])
                kt = pool.tile([128, CW * G], F32, tag="kt")
                nc.vector.scalar_tensor_tensor(kt[:], sb["klane"], float(i0), m[:], ALU.add, ALU.mult)
                nc.vector.select(runidxw[:], m[:], kt[:], runidxw[:])
                # per-anchor max over g and argmax
                nc.vector.tensor_reduce(maxsh[:, S], _v(s2[:]), mybir.AxisListType.X, ALU.max)
                rmbv = maxsh[:, S].rearrange("p (c o) -> p c o", o=1).broadcast_to((128, CW, G))
                elig = pool.tile([128, CW * G], F32, tag="elig")
                nc.vector.tensor_tensor(_v(elig[:]), _v(s2[:]), rmbv, ALU.is_ge)
                nc.vector.tensor_scalar(elig[:], elig[:], -1e9, 1e9, ALU.mult, ALU.add)
                nc.vector.tensor_tensor(_v(elig[:]), _v(elig[:]), gidxv, ALU.add)
                am = pool.tile([128, CW], F32, tag="am")
                nc.vector.tensor_reduce(am[:], _v(elig[:]), mybir.AxisListType.X, ALU.min)
                ambv = am[:].rearrange("p (c o) -> p c o", o=1).broadcast_to((128, CW, G))
                oh = pool.tile([128, CW * G], F32, tag="oh")
                nc.vector.tensor_tensor(_v(oh[:]), gidxv, ambv, ALU.is_equal)
                # gather gt coords: g4[:, S, j] = sum_g oh * coord_rep
                for j, cr in enumerate(("gx1r", "gy1r", "gx2r", "gy2r")):
                    pr = pool.tile([128, CW * G], F32, tag=f"pr{j}")
                    nc.vector.tensor_tensor(pr[:], oh[:], sb[cr], ALU.mult)
                    nc.vector.tensor_reduce(g4[:, S, j], _v(pr[:]), mybir.AxisListType.X, ALU.add)

            # ---- phase C: labels ----
            lb1 = pool.tile([128, K], F32, tag="lb1")
            lb2 = pool.tile([128, K], F32, tag="lb2")
            nc.vector.tensor_scalar(lb1[:], maxsh[:], 1.0 + NEG_OV, None, ALU.is_lt)
            nc.vector.tensor_scalar(lb2[:], maxsh[:], 1.0 + POS_OV, None, ALU.is_ge)
            nc.vector.scalar_tensor_tensor(lb1[:], lb2[:], 2.0, lb1[:], ALU.mult, ALU.add)
            nc.vector.tensor_tensor(lb1[:], lb1[:], inside[:], ALU.mult)
            nc.vector.tensor_scalar(outt[:, OCOLS["lab"][0]:OCOLS["lab"][0] + K], lb1[:], -1.0, None, ALU.add)
            # ---- phase C: targets ----
            gx1g, gy1g = g4[:, :, 0], g4[:, :, 1]
            gx2g, gy2g = g4[:, :, 2], g4[:, :, 3]
            t4 = outt[:, K:5 * K].rearrange("p (k f) -> p k f", f=4)
            gw = pool.tile([128, K], F32, tag="gw")
            gh = pool.tile([128, K], F32, tag="gh")
            d1 = pool.tile([128, K], F32, tag="d1")
            d2 = pool.tile([128, K], F32, tag="d2")
            nc.vector.scalar_tensor_tensor(gw[:], gx2g, 1.0, gx1g, ALU.add, ALU.subtract)
            nc.vector.scalar_tensor_tensor(gh[:], gy2g, 1.0, gy1g, ALU.add, ALU.subtract)
            # tx
            nc.vector.tensor_tensor(d1[:], gx1g, ax1, ALU.subtract)
            nc.vector.tensor_tensor(d2[:], gw[:], aw[:], ALU.subtract)
            nc.vector.scalar_tensor_tensor(d1[:], d2[:], 0.5, d1[:], ALU.mult, ALU.add)
            nc.vector.tensor_tensor(d1[:], d1[:], raw[:], ALU.mult)
            nc.vector.tensor_tensor(t4[:, :, 0], d1[:], inside[:], ALU.mult)
            # ty
            nc.vector.tensor_tensor(d1[:], gy1g, ay1, ALU.subtract)
            nc.vector.tensor_tensor(d2[:], gh[:], ah[:], ALU.subtract)
            nc.vector.scalar_tensor_tensor(d1[:], d2[:], 0.5, d1[:], ALU.mult, ALU.add)
            nc.vector.tensor_tensor(d1[:], d1[:], rah[:], ALU.mult)
            nc.vector.tensor_tensor(t4[:, :, 1], d1[:], inside[:], ALU.mult)
            # tw, th
            nc.vector.tensor_tensor(t4[:, :, 2], gw[:], raw[:], ALU.mult)
            nc.vector.tensor_tensor(t4[:, :, 3], gh[:], rah[:], ALU.mult)
            
            # ---- phase C: fold per-gt runmax over the CW lanes ----
            rmv = runmaxw[:].rearrange("p (c g) -> p g c", c=CW)
            rix_v = runidxw[:].rearrange("p (c g) -> p g c", c=CW)
            rmo = outt[:, OCOLS["rmx"][0]:OCOLS["rmx"][0] + G]
            nc.vector.tensor_reduce(rmo, rmv, mybir.AxisListType.X, ALU.max)
            mbv = rmo.rearrange("p (g o) -> p g o", o=1).broadcast_to((128, G, CW))
            el2 = pool.tile([128, CW * G], F32, tag="el2")
            el2v = el2[:].rearrange("p (c g) -> p g c", c=CW)
            nc.vector.tensor_tensor(el2v, rmv, mbv, ALU.is_ge)
            nc.vector.tensor_scalar(el2[:], el2[:], -1e9, 1e9, ALU.mult, ALU.add)
            nc.vector.tensor_tensor(el2[:], el2[:], runidxw[:], ALU.add)
            rio = outt[:, OCOLS["rix"][0]:OCOLS["rix"][0] + G]
            nc.vector.tensor_reduce(rio, el2v, mybir.AxisListType.X, ALU.min)
            nc.sync.dma_start(outp[:], outt[:])
    return nc


def _subsample(labels, target_val, max_keep, u):
    mask = labels == target_val
    n = int(mask.sum())
    scores = np.where(mask, u, np.float32(2.0))
    order = np.argsort(scores, kind="stable")
    ranks = np.empty(N, dtype=np.int64)
    ranks[order] = np.arange(N)
    drop = mask & (ranks >= max_keep) & (n > max_keep)
    labels = labels.copy()
    labels[drop] = -1.0
    return labels


def kernel(scores, gt_boxes, metadata):
    scores = np.asarray(scores); gt_boxes = np.asarray(gt_boxes); metadata = np.asarray(metadata)
    anch = _all_anchors()                       # (N,4) f32 constant
    gt = np.asarray(gt_boxes[0], dtype=np.float32)  # (G,4)
    meta = np.asarray(metadata[0], dtype=np.float32)
    nc = _build_nc()

    area_g = ((gt[:, 2] - gt[:, 0] + 1.0) * (gt[:, 3] - gt[:, 1] + 1.0)).astype(np.float32)
    aw_h = anch[:, 2] - anch[:, 0] + 1.0
    ah_h = anch[:, 3] - anch[:, 1] + 1.0
    area_a = (aw_h * ah_h).astype(np.float32)

    def plane(vec_nc):  # (NC,) -> (128, K) with [p,k] = v[k*128+p]
        return np.ascontiguousarray(vec_nc.reshape(K, 128).T)

    gidxw = np.tile(np.arange(G, dtype=np.float32), (128, CW))
    klane = np.tile(np.repeat(np.arange(CW, dtype=np.float32), G), (128, 1))
    # blockdiag: B[c*64+g, c*4+j] = gt[g, j]; fed as (128, 4*32) with column
    # block j holding rows [j*128, (j+1)*128) of B
    Bfull = np.zeros((CW * G, CW * 4), dtype=np.float32)
    for c in range(CW):
        Bfull[c * G:(c + 1) * G, c * 4:(c + 1) * 4] = gt
    bmat_in = np.ascontiguousarray(
        np.concatenate([Bfull[j * 128:(j + 1) * 128, :] for j in range(4)], axis=1))

    common = {
        "gx1r": np.tile(gt[:, 0], (128, CW)).astype(np.float32),
        "gy1r": np.tile(gt[:, 1], (128, CW)).astype(np.float32),
        "gx2r": np.tile(gt[:, 2], (128, CW)).astype(np.float32),
        "gy2r": np.tile(gt[:, 3], (128, CW)).astype(np.float32),
        "areagr": np.tile(area_g, (128, CW)).astype(np.float32),
        "gidxw": gidxw.astype(np.float32),
        "klane": klane.astype(np.float32),
        "meta2": np.tile(np.array([meta[1], meta[0]], dtype=np.float32), (128, 1)),
    }
    in_maps = []
    for c in range(NCORES):
        sl = slice(c * NC, (c + 1) * NC)
        m = dict(common)
        m["ax1"] = plane(anch[sl, 0]); m["ay1"] = plane(anch[sl, 1])
        m["ax2"] = plane(anch[sl, 2]); m["ay2"] = plane(anch[sl, 3])
        m["areaa"] = plane(area_a[sl])
        order = ["ax1", "ay1", "ax2", "ay2", "areaa", "gx1r", "gy1r", "gx2r", "gy2r",
                 "areagr", "gidxw", "klane", "meta2"]
        in_maps.append({"inp": np.ascontiguousarray(np.concatenate([m[k] for k in order], axis=1))})

    inside_full = ((anch[:, 0] >= 0) & (anch[:, 1] >= 0) &
                   (anch[:, 2] < meta[1]) & (anch[:, 3] < meta[0]))
    global LAST_EXEC_NS
    try:
        if globals().get("TRACE", False):
            res = run_bass_kernel_spmd(nc, in_maps, core_ids=list(range(NCORES)), trace=True)
        else:
            raise RuntimeError("no trace")
    except Exception:
        res = run_bass_kernel_spmd(nc, in_maps, core_ids=list(range(NCORES)))
    LAST_EXEC_NS = getattr(res, "exec_time_ns", None)
    labs, tgts, rmxs, rixs = [], [], [], []
    for c in range(NCORES):
        o = np.asarray(res.results[c]["out"])
        labs.append(o[:, 0:K].transpose(1, 0).reshape(-1))
        t = o[:, K:5 * K].reshape(128, K, 4).transpose(1, 0, 2).reshape(-1, 4).copy()
        ins_m = inside_full[c * NC:(c + 1) * NC]
        t[:, 2] = np.where(ins_m, np.log(np.where(ins_m, t[:, 2], 1.0)), 0.0)
        t[:, 3] = np.where(ins_m, np.log(np.where(ins_m, t[:, 3], 1.0)), 0.0)
        tgts.append(t)
        rmxs.append(o[:, 5 * K:5 * K + G])
        rixs.append(o[:, 5 * K + G:5 * K + 2 * G])
    labels = np.concatenate(labs).astype(np.float32)
    targets = np.concatenate(tgts, axis=0).astype(np.float32)

    # per-gt global argmax (value desc, then index asc) from per-core partials
    rmx = np.stack(rmxs)  # (8,128,G) shifted-masked max per (core, p, g)
    rix = np.stack(rixs)  # chunk index k of that max
    for g in range(G):
        v = rmx[:, :, g]
        M = v.max()
        cores, ps = np.nonzero(v >= M)
        ks = rix[cores, ps, g]
        n_glob = cores * NC + ks.astype(np.int64) * 128 + ps
        labels[int(n_glob.min())] = 1.0

    u_pos, u_neg = _uniforms()
    num_fg = int(RPN_FG_FRACTION * RPN_BATCHSIZE)
    labels = _subsample(labels, 1.0, num_fg, u_pos)
    num_bg = RPN_BATCHSIZE - int((labels == 1.0).sum())
    labels = _subsample(labels, 0.0, num_bg, u_neg)

    return anch[None], labels[None], targets[None]


# revision 21
# speedup vs baseline: 1.2160x; 1.2160x over previous
import sys, functools
import numpy as np

sys.path.insert(0, "/opt/trn_rl_repo")

import concourse.bass as bass
import concourse.bacc as bacc
import concourse.mybir as mybir
from concourse import tile
from concourse.vector_clock import ScopedClock


class _TC(tile.TileContext):
    # the local walrus build allows at most ONE sync-wait per instruction;
    # split the kernel-tail drain's waits across single-wait NOPs
    def _drain_and_barrier(self, tick_clock, wait_clock):
        nc = self.nc
        probe = nc.sync.nop()
        wait_clock.add_sem_waits(probe.ins, ScopedClock({None: tick_clock.global_clock}))
        si = probe.ins.sync_info
        ws = list(si.on_wait) if si and si.on_wait else []
        if len(ws) > 1:
            import concourse.mybir as mybir
            probe.ins.sync_info = mybir.SyncInfo(
                on_wait=[ws[0]], on_update=list(si.on_update or []))
            for w in ws[1:]:
                n2 = nc.sync.nop()
                n2.ins.sync_info = mybir.SyncInfo(on_wait=[w], on_update=[])
        nc.sync.drain()
        nc.all_engine_barrier()
        popped = nc._tile_sem_poison_stack.pop()
        assert popped is self._sem_poison
        nc.clear_and_free_semaphores(list(self.sems.allocated().values()))
        nc.all_engine_barrier()
from concourse.bass_utils import run_bass_kernel_spmd

# ---- problem constants (hardcoded per spec) ----
RR, CC, A, G = 256, 256, 9, 64
N = RR * CC * A            # 589824
NCORES = 8
NC = N // NCORES           # 73728 anchors per core
K = NC // 128              # 576 chunk-columns per core
CW = 8                     # chunks packed per wide tile
NIT = K // CW              # 72 iterations
STRIDE = 16
NEG_OV, POS_OV = 0.3, 0.7
RPN_BATCHSIZE, RPN_FG_FRACTION = 256, 0.5
F32 = mybir.dt.float32
ALU = mybir.AluOpType
ACTF = mybir.ActivationFunctionType


def _base_anchors():
    base = np.array([1, 1, 16, 16], dtype=np.float64) - 1
    w = base[2] - base[0] + 1
    h = base[3] - base[1] + 1
    x_ctr = base[0] + 0.5 * (w - 1)
    y_ctr = base[1] + 0.5 * (h - 1)
    size = w * h
    out = []
    for r in (0.5, 1.0, 2.0):
        ws = np.round(np.sqrt(size / r))
        hs = np.round(ws * r)
        for s in (8, 16, 32):
            wss, hss = ws * s, hs * s
            out.append([x_ctr - 0.5 * (wss - 1), y_ctr - 0.5 * (hss - 1),
                        x_ctr + 0.5 * (wss - 1), y_ctr + 0.5 * (hss - 1)])
    return np.array(out, dtype=np.float32)


@functools.lru_cache(maxsize=1)
def _all_anchors():
    base = _base_anchors()
    sx, sy = np.meshgrid(np.arange(CC, dtype=np.float32) * STRIDE,
                         np.arange(RR, dtype=np.float32) * STRIDE)
    shifts = np.stack([sx.ravel(), sy.ravel(), sx.ravel(), sy.ravel()], axis=1).astype(np.float32)
    return (base[None, :, :] + shifts[:, None, :]).reshape(-1, 4)  # (N,4) f32


@functools.lru_cache(maxsize=1)
def _uniforms():
    import jax
    with jax.default_device(jax.devices("cpu")[0]):
        key = jax.random.key(1)
        k_pos, k_neg = jax.random.split(key)
        u_pos = np.asarray(jax.random.uniform(k_pos, (N,)), dtype=np.float32)
        u_neg = np.asarray(jax.random.uniform(k_neg, (N,)), dtype=np.float32)
    return u_pos, u_neg


def _v(ap):  # (128, CW*G) -> (128, CW, G)
    return ap.rearrange("p (c g) -> p c g", c=CW)


@functools.lru_cache(maxsize=1)
def _build_nc():
    nc = bass.Bass()
    COLS = {}
    off = 0
    for nm, w in [("ax1", K), ("ay1", K), ("ax2", K), ("ay2", K), ("areaa", K),
                  ("gx1r", CW * G), ("gy1r", CW * G), ("gx2r", CW * G), ("gy2r", CW * G),
                  ("areagr", CW * G), ("gidxw", CW * G), ("klane", CW * G), ("meta2", 2)]:
        COLS[nm] = (off, w); off += w
    INW = off
    OCOLS = {"lab": (0, K), "tgt": (K, 4 * K), "rmx": (5 * K, G), "rix": (5 * K + G, G)}
    OUTW = 5 * K + 2 * G
    inp = nc.declare_dram_parameter("inp", [128, INW], F32, isOutput=False)
    outp = nc.declare_dram_parameter("out", [128, OUTW], F32, isOutput=True)
    with _TC(nc) as tc:
        with tc.tile_pool(name="main", bufs=2) as pool, \
             tc.tile_pool(name="cons", bufs=1) as cpool, \
             tc.tile_pool(name="ps", bufs=2, space="PSUM") as ppool:
            # load inputs to SBUF (single DMA)
            inpt = cpool.tile([128, INW], F32, tag="inpt")
            nc.sync.dma_start(inpt[:], inp[:])
            nc.vector.tensor_scalar(inpt[:], inpt[:], 0.0, None, ALU.add)
            sb = {nm: inpt[:, o:o + w] for nm, (o, w) in COLS.items()}
            outt = cpool.tile([128, OUTW], F32, tag="outt")
            ax1, ay1, ax2, ay2 = sb["ax1"], sb["ay1"], sb["ax2"], sb["ay2"]
            areaa, inside = sb["areaa"], cpool.tile([128, K], F32, tag="inside")
            # ---- phase A: inside mask, anchor widths ----
            c1 = pool.tile([128, K], F32, tag="pA")
            c2 = pool.tile([128, K], F32, tag="pB")
            nc.vector.tensor_scalar(c1[:], ax1, 0.0, None, ALU.is_ge)
            nc.vector.tensor_scalar(c2[:], ay1, 0.0, None, ALU.is_ge)
            nc.vector.tensor_tensor(inside[:], c1[:], c2[:], ALU.mult)
            nc.vector.tensor_scalar(c1[:], ax2, sb["meta2"][:, 0:1], None, ALU.is_lt)
            nc.vector.tensor_scalar(c2[:], ay2, sb["meta2"][:, 1:2], None, ALU.is_lt)
            nc.vector.tensor_tensor(c1[:], c1[:], c2[:], ALU.mult)
            nc.vector.tensor_tensor(inside[:], inside[:], c1[:], ALU.mult)
            aw = cpool.tile([128, K], F32, tag="aw")
            ah = cpool.tile([128, K], F32, tag="ah")
            raw = cpool.tile([128, K], F32, tag="raw")
            rah = cpool.tile([128, K], F32, tag="rah")
            nc.vector.scalar_tensor_tensor(aw[:], ax2, 1.0, ax1, ALU.add, ALU.subtract)
            nc.vector.scalar_tensor_tensor(ah[:], ay2, 1.0, ay1, ALU.add, ALU.subtract)
            nc.vector.reciprocal(raw[:], aw[:])
            nc.vector.reciprocal(rah[:], ah[:])

            # ---- bulk result tiles ----
            maxsh = cpool.tile([128, K], F32, tag="maxsh")
            g4 = cpool.tile([128, K, 4], F32, tag="g4")
            runmaxw = cpool.tile([128, CW * G], F32, tag="runmaxw")
            runidxw = cpool.tile([128, CW * G], F32, tag="runidxw")
            nc.vector.memset(runmaxw[:], 0.0)
            nc.vector.memset(runidxw[:], 0.0)
            gx1v, gy1v = _v(sb["gx1r"]), _v(sb["gy1r"])
            gx2v, gy2v = _v(sb["gx2r"]), _v(sb["gy2r"])
            areagv, gidxv = _v(sb["areagr"]), _v(sb["gidxw"])

            # ---- phase B: main loop over 72 wide tiles ----
            for it in range(NIT):
                i0 = it * CW
                S = slice(i0, i0 + CW)
                def bc(plane):  # (128,CW) cols -> (128,CW,G) bcast
                    return plane[:, S].rearrange("p (c o) -> p c o", o=1).broadcast_to((128, CW, G))
                t1 = pool.tile([128, CW * G], F32, tag="t1")
                t2 = pool.tile([128, CW * G], F32, tag="t2")
                iwp = pool.tile([128, CW * G], F32, tag="iwp")
                ihp = pool.tile([128, CW * G], F32, tag="ihp")
                nc.vector.tensor_tensor(_v(t1[:]), gx2v, bc(ax2), ALU.min)
                nc.vector.tensor_tensor(_v(t2[:]), gx1v, bc(ax1), ALU.max)
                nc.vector.scalar_tensor_tensor(t1[:], t1[:], 1.0, t2[:], ALU.add, ALU.subtract)
                nc.scalar.activation(iwp[:], t1[:], ACTF.Relu)
                t3 = pool.tile([128, CW * G], F32, tag="t3")
                t4 = pool.tile([128, CW * G], F32, tag="t4")
                nc.vector.tensor_tensor(_v(t3[:]), gy2v, bc(ay2), ALU.min)
                nc.vector.tensor_tensor(_v(t4[:]), gy1v, bc(ay1), ALU.max)
                nc.vector.scalar_tensor_tensor(t3[:], t3[:], 1.0, t4[:], ALU.add, ALU.subtract)
                nc.scalar.activation(ihp[:], t3[:], ACTF.Relu)
                inter = pool.tile([128, CW * G], F32, tag="inter")
                nc.vector.tensor_tensor(inter[:], iwp[:], ihp[:], ALU.mult)
                un = pool.tile([128, CW * G], F32, tag="un")
                nc.vector.tensor_tensor(_v(un[:]), areagv, bc(areaa), ALU.add)
                nc.vector.tensor_tensor(un[:], un[:], inter[:], ALU.subtract)
                rec = pool.tile([128, CW * G], F32, tag="rec")
                nc.scalar.activation(rec[:], un[:], ACTF.Ln)
                nc.scalar.activation(rec[:], rec[:], ACTF.Exp, bias=0.0, scale=-1.0)
                iou = pool.tile([128, CW * G], F32, tag="iou")
                nc.vector.tensor_tensor(iou[:], inter[:], rec[:], ALU.mult)
                s2 = pool.tile([128, CW * G], F32, tag="s2")
                nc.vector.tensor_scalar(iou[:], iou[:], 1.0, None, ALU.add)
                nc.vector.tensor_tensor(_v(s2[:]), _v(iou[:]), bc(inside[:]), ALU.mult)
                # per-gt running argmax
                m = pool.tile([128, CW * G], mybir.dt.uint8, tag="m")
                nc.vector.tensor_tensor(m[:], s2[:], runmaxw[:], ALU.is_gt)
                nc.vector.select(runmaxw[:], m[:], s2[:], runmaxw[:])
                kt = pool.tile([128, CW * G], F32, tag="kt")
                nc.vector.scalar_tensor_tensor(kt[:], sb["klane"], float(i0), m[:], ALU.add, ALU.mult)
                nc.vector.select(runidxw[:], m[:], kt[:], runidxw[:])
                # per-anchor max over g and argmax
                nc.vector.tensor_reduce(maxsh[:, S], _v(s2[:]), mybir.AxisListType.X, ALU.max)
                rmbv = maxsh[:, S].rearrange("p (c o) -> p c o", o=1).broadcast_to((128, CW, G))
                elig = pool.tile([128, CW * G], F32, tag="elig")
                nc.vector.tensor_tensor(_v(elig[:]), _v(s2[:]), rmbv, ALU.is_ge)
                nc.vector.tensor_scalar(elig[:], elig[:], -1e9, 1e9, ALU.mult, ALU.add)
                nc.vector.tensor_tensor(_v(elig[:]), _v(elig[:]), gidxv, ALU.add)
                am = pool.tile([128, CW], F32, tag="am")
                nc.vector.tensor_reduce(am[:], _v(elig[:]), mybir.AxisListType.X, ALU.min)
                ambv = am[:].rearrange("p (c o) -> p c o", o=1).broadcast_to((128, CW, G))
                oh = pool.tile([128, CW * G], F32, tag="oh")
                nc.vector.tensor_tensor(_v(oh[:]), gidxv, ambv, ALU.is_equal)
                # gather gt coords: g4[:, S, j] = sum_g oh * coord_rep
                for j, cr in enumerate(("gx1r", "gy1r", "gx2r", "gy2r")):
                    pr = pool.tile([128, CW * G], F32, tag=f"pr{j}")
                    nc.vector.tensor_tensor(pr[:], oh[:], sb[cr], ALU.mult)
                    nc.vector.tensor_reduce(g4[:, S, j], _v(pr[:]), mybir.AxisListType.X, ALU.add)

            # ---- phase C: labels ----
            lb1 = pool.tile([128, K], F32, tag="lb1")
            lb2 = pool.tile([128, K], F32, tag="lb2")
            nc.vector.tensor_scalar(lb1[:], maxsh[:], 1.0 + NEG_OV, None, ALU.is_lt)
            nc.vector.tensor_scalar(lb2[:], maxsh[:], 1.0 + POS_OV, None, ALU.is_ge)
            nc.vector.scalar_tensor_tensor(lb1[:], lb2[:], 2.0, lb1[:], ALU.mult, ALU.add)
            nc.vector.tensor_tensor(lb1[:], lb1[:], inside[:], ALU.mult)
            nc.vector.tensor_scalar(outt[:, OCOLS["lab"][0]:OCOLS["lab"][0] + K], lb1[:], -1.0, None, ALU.add)
            # ---- phase C: targets ----
            gx1g, gy1g = g4[:, :, 0], g4[:, :, 1]
            gx2g, gy2g = g4[:, :, 2], g4[:, :, 3]
            t4 = outt[:, K:5 * K].rearrange("p (k f) -> p k f", f=4)
            gw = pool.tile([128, K], F32, tag="gw")
            gh = pool.tile([128, K], F32, tag="gh")
            d1 = pool.tile([128, K], F32, tag="d1")
            d2 = pool.tile([128, K], F32, tag="d2")
            nc.vector.scalar_tensor_tensor(gw[:], gx2g, 1.0, gx1g, ALU.add, ALU.subtract)
            nc.vector.scalar_tensor_tensor(gh[:], gy2g, 1.0, gy1g, ALU.add, ALU.subtract)
            # tx
            nc.vector.tensor_tensor(d1[:], gx1g, ax1, ALU.subtract)
            nc.vector.tensor_tensor(d2[:], gw[:], aw[:], ALU.subtract)
            nc.vector.scalar_tensor_tensor(d1[:], d2[:], 0.5, d1[:], ALU.mult, ALU.add)
            nc.vector.tensor_tensor(d1[:], d1[:], raw[:], ALU.mult)
            nc.vector.tensor_tensor(t4[:, :, 0], d1[:], inside[:], ALU.mult)
            # ty
            nc.vector.tensor_tensor(d1[:], gy1g, ay1, ALU.subtract)
            nc.vector.tensor_tensor(d2[:], gh[:], ah[:], ALU.subtract)
            nc.vector.scalar_tensor_tensor(d1[:], d2[:], 0.5, d1[:], ALU.mult, ALU.add)
            nc.vector.tensor_tensor(d1[:], d1[:], rah[:], ALU.mult)
            nc.vector.tensor_tensor(t4[:, :, 1], d1[:], inside[:], ALU.mult)
            # tw, th
            nc.vector.tensor_tensor(t4[:, :, 2], gw[:], raw[:], ALU.mult)
            nc.vector.tensor_tensor(t4[:, :, 3], gh[:], rah[:], ALU.mult)
            
            # ---- phase C: fold per-gt runmax over the CW lanes ----
            rmv = runmaxw[:].rearrange("p (c g) -> p g c", c=CW)
            rix_v = runidxw[:].rearrange("p (c g) -> p g c", c=CW)
            rmo = outt[:, OCOLS["rmx"][0]:OCOLS["rmx"][0] + G]
            nc.vector.tensor_reduce(rmo, rmv, mybir.AxisListType.X, ALU.max)
            mbv = rmo.rearrange("p (g o) -> p g o", o=1).broadcast_to((128, G, CW))
            el2 = pool.tile([128, CW * G], F32, tag="el2")
            el2v = el2[:].rearrange("p (c g) -> p g c", c=CW)
            nc.vector.tensor_tensor(el2v, rmv, mbv, ALU.is_ge)
            nc.vector.tensor_scalar(el2[:], el2[:], -1e9, 1e9, ALU.mult, ALU.add)
            nc.vector.tensor_tensor(el2[:], el2[:], runidxw[:], ALU.add)
            rio = outt[:, OCOLS["rix"][0]:OCOLS["rix"][0] + G]
            nc.vector.tensor_reduce(rio, el2v, mybir.AxisListType.X, ALU.min)
            nc.sync.dma_start(outp[:], outt[:])
    return nc


def _subsample(labels, target_val, max_keep, u):
    mask = labels == target_val
    n = int(mask.sum())
    scores = np.where(mask, u, np.float32(2.0))
    order = np.argsort(scores, kind="stable")
    ranks = np.empty(N, dtype=np.int64)
    ranks[order] = np.arange(N)
    drop = mask & (ranks >= max_keep) & (n > max_keep)
    labels = labels.copy()
    labels[drop] = -1.0
    return labels


def kernel(scores, gt_boxes, metadata):
    scores = np.asarray(scores); gt_boxes = np.asarray(gt_boxes); metadata = np.asarray(metadata)
    anch = _all_anchors()                       # (N,4) f32 constant
    gt = np.asarray(gt_boxes[0], dtype=np.float32)  # (G,4)
    meta = np.asarray(metadata[0], dtype=np.float32)
    nc = _build_nc()

    area_g = ((gt[:, 2] - gt[:, 0] + 1.0) * (gt[:, 3] - gt[:, 1] + 1.0)).astype(np.float32)
    aw_h = anch[:, 2] - anch[:, 0] + 1.0
    ah_h = anch[:, 3] - anch[:, 1] + 1.0
    area_a = (aw_h * ah_h).astype(np.float32)

    def plane(vec_nc):  # (NC,) -> (128, K) with [p,k] = v[k*128+p]
        return np.ascontiguousarray(vec_nc.reshape(K, 128).T)

    gidxw = np.tile(np.arange(G, dtype=np.float32), (128, CW))
    klane = np.tile(np.repeat(np.arange(CW, dtype=np.float32), G), (128, 1))
    # blockdiag: B[c*64+g, c*4+j] = gt[g, j]; fed as (128, 4*32) with column
    # block j holding rows [j*128, (j+1)*128) of B
    Bfull = np.zeros((CW * G, CW * 4), dtype=np.float32)
    for c in range(CW):
        Bfull[c * G:(c + 1) * G, c * 4:(c + 1) * 4] = gt
    bmat_in = np.ascontiguousarray(
        np.concatenate([Bfull[j * 128:(j + 1) * 128, :] for j in range(4)], axis=1))

    common = {
        "gx1r": np.tile(gt[:, 0], (128, CW)).astype(np.float32),
        "gy1r": np.tile(gt[:, 1], (128, CW)).astype(np.float32),
        "gx2r": np.tile(gt[:, 2], (128, CW)).astype(np.float32),
        "gy2r": np.tile(gt[:, 3], (128, CW)).astype(np.float32),
        "areagr": np.tile(area_g, (128, CW)).astype(np.float32),
        "gidxw": gidxw.astype(np.float32),
        "klane": klane.astype(np.float32),
        "meta2": np.tile(np.array([meta[1], meta[0]], dtype=np.float32), (128, 1)),
    }
    in_maps = []
    for c in range(NCORES):
        sl = slice(c * NC, (c + 1) * NC)
        m = dict(common)
        m["ax1"] = plane(anch[sl, 0]); m["ay1"] = plane(anch[sl, 1])
        m["ax2"] = plane(anch[sl, 2]); m["ay2"] = plane(anch[sl, 3])
        m["areaa"] = plane(area_a[sl])
        order = ["ax1", "ay1", "ax2", "ay2", "areaa", "gx1r", "gy1r", "gx2r", "gy2r",
                 "areagr", "gidxw", "klane", "meta2"]
        in_maps.append({"inp": np.ascontiguousarray(np.concatenate([m[k] for k in order], axis=1))})

    inside_full = ((anch[:, 0] >= 0) & (anch[:, 1] >= 0) &
                   (anch[:, 2] < meta[1]) & (anch[:, 3] < meta[0]))
    global LAST_EXEC_NS
    try:
        if globals().get("TRACE", False):
            res = run_bass_kernel_spmd(nc, in_maps, core_ids=list(range(NCORES)), trace=True)
        else:
            raise RuntimeError("no trace")
    except Exception:
        res = run_bass_kernel_spmd(nc, in_maps, core_ids=list(range(NCORES)))
    LAST_EXEC_NS = getattr(res, "exec_time_ns", None)
    labs, tgts, rmxs, rixs = [], [], [], []
    for c in range(NCORES):
        o = np.asarray(res.results[c]["out"])
        labs.append(o[:, 0:K].transpose(1, 0).reshape(-1))
        t = o[:, K:5 * K].reshape(128, K, 4).transpose(1, 0, 2).reshape(-1, 4).copy()
        ins_m = inside_full[c * NC:(c + 1) * NC]
        t[:, 2] = np.where(ins_m, np.log(np.where(ins_m, t[:, 2], 1.0)), 0.0)
        t[:, 3] = np.where(ins_m, np.log(np.where(ins_m, t[:, 3], 1.0)), 0.0)
        tgts.append(t)
        rmxs.append(o[:, 5 * K:5 * K + G])
        rixs.append(o[:, 5 * K + G:5 * K + 2 * G])
    labels = np.concatenate(labs).astype(np.float32)
    targets = np.concatenate(tgts, axis=0).astype(np.float32)

    # per-gt global argmax (value desc, then index asc) from per-core partials
    rmx = np.stack(rmxs)  # (8,128,G) shifted-masked max per (core, p, g)
    rix = np.stack(rixs)  # chunk index k of that max
    for g in range(G):
        v = rmx[:, :, g]
        M = v.max()
        cores, ps = np.nonzero(v >= M)
        ks = rix[cores, ps, g]
        n_glob = cores * NC + ks.astype(np.int64) * 128 + ps
        labels[int(n_glob.min())] = 1.0

    u_pos, u_neg = _uniforms()
    num_fg = int(RPN_FG_FRACTION * RPN_BATCHSIZE)
    labels = _subsample(labels, 1.0, num_fg, u_pos)
    num_bg = RPN_BATCHSIZE - int((labels == 1.0).sum())
    labels = _subsample(labels, 0.0, num_bg, u_neg)

    return anch[None], labels[None], targets[None]
